# revision 6
# baseline (speedup 1.0000x reference)
"""GATConv (PyG defaults: add_self_loops, concat=False/head-mean) on 8 Trainium2 cores.

v2 strategy — aggregate in x-space, gather 256-B rows, batch all small math:

Edges are bucketed by DESTINATION node. Core k owns nodes [k*NPC, (k+1)*NPC)
and every edge whose dst lands there; segment softmax and aggregation are
core-local (no collectives). Host concatenates the 8 output slices.

Key insight vs v1: out[dst] = (sum_e alpha_e * x[src_e]) @ W per head — the
W matmul is LINEAR, so aggregate raw x (256 B/edge gather instead of 1280 B)
and apply W once per dst block. Per-edge attention logits a_s[src]+a_d[dst]
are shipped from host as flat arrays (pure indexing; the exp/max/normalize
math stays on device and is batched over ALL edges in a handful of large
instructions instead of per-tile ops).

Device program (SPMD-identical; per-core data via host-supplied arrays):
  Phase 0: stage x-table (f16, [NPAD,128]) into Internal DRAM (gather ucode
      reads base addresses directly, so the source must be Internal); batched
      edge math: z = as+ad; ev = max(exp(z-C), exp(.2z-C)) (= exp(lrelu(z)-C),
      C host-picked so ev fits f16); ev16 = cast(ev).
  Phase 1: per dst block b (49/core): psum_agg[dst,4*128] accumulates one-hot
      scatter matmuls S01^T @ (ev ⊙ x_src) over the block's edge tiles;
      psum_den[dst,4] accumulates S01^T @ ev16. Self-loop edges form a
      dedicated first tile with S01 = identity and x loaded sequentially (no
      gather). Non-self edges are gathered from the x-table via dma_gather
      (int16 idx => table split in two halves; tiles grouped (block-pair,
      half) to halve call count).
      Tail: agg/(4*den) -> f16, PE-transpose per head, aggT @ W_h accumulated
      over heads -> out rows.
"""

import math
import sys

import numpy as np

if "/opt/trn_rl_repo" not in sys.path:
    sys.path.insert(0, "/opt/trn_rl_repo")

P = 128
SLOPE = 0.2
ZPAD = -1.0e4          # logit for padding slots -> ev == 0


class Cfg:
    def __init__(self, N=50000, E=800000, DIN=128, DOUT=128, H=4, ncores=8):
        self.N, self.E, self.DIN, self.DOUT, self.H = N, E, DIN, DOUT, H
        self.NCORES = ncores
        self.NPC = N // ncores                 # nodes per core
        self.NBLK = math.ceil(self.NPC / P)    # dst blocks per core
        self.LAST_ROWS = self.NPC - (self.NBLK - 1) * P
        self.NPAD = math.ceil(N / P) * P       # padded node count
        self.NTILE_N = self.NPAD // P
        self.NSPLIT = self.NPAD // 2           # x-table half split row
        self.WH = H * DOUT                     # 512
        assert DIN == P and DOUT == P and self.WH == 512
        assert self.NSPLIT < 32768 and self.NPAD - self.NSPLIT < 32768


DEFAULT_CFG = Cfg()


def _build_program(cfg: Cfg, t_half: int):
    from contextlib import ExitStack

    import concourse.bacc as bacc
    import concourse.bass as bass
    import concourse.mybir as mybir
    import concourse.tile as tile

    f16 = mybir.dt.float16
    f32 = mybir.dt.float32
    i16 = mybir.dt.int16
    AF = mybir.ActivationFunctionType
    H, DOUT, WH = cfg.H, cfg.DOUT, cfg.WH
    NBLK = cfg.NBLK
    TB = 1 + 2 * t_half            # tiles per block (tile 0 = self loops)
    nt = NBLK * TB                 # total tiles per core
    NPAIR = NBLK // 2              # full block pairs
    NIH2 = 2 * t_half * P          # idx per paired gather call
    NIH1 = t_half * P              # idx per leftover-block call
    NGI = (nt - NBLK) * 8          # hidx columns ([P, NGI] wrap16 layout)

    nc = bacc.Bacc(
        "TRN2",
        target_bir_lowering=False,
        debug=False,
        enable_asserts=False,
        num_devices=cfg.NCORES,
    )

    xtab_in = nc.dram_tensor("xtab", [cfg.NPAD, P], f16, kind="ExternalInput").ap()
    iota_in = nc.dram_tensor("iota", [P, P], f16, kind="ExternalInput").ap()
    ident_in = nc.dram_tensor("ident", [P, P], f16, kind="ExternalInput").ap()
    w_in = nc.dram_tensor("w", [P, WH], f16, kind="ExternalInput").ap()
    bias_in = nc.dram_tensor("bias", [P, 1], f32, kind="ExternalInput").ap()
    as_in = nc.dram_tensor("ase", [P, nt * H], f32, kind="ExternalInput").ap()
    ad_in = nc.dram_tensor("ade", [P, nt * H], f32, kind="ExternalInput").ap()
    hidx_in = nc.dram_tensor("hidx", [P, NGI], i16, kind="ExternalInput").ap()
    dlocv_in = nc.dram_tensor("dlocv", [P, nt], f16, kind="ExternalInput").ap()
    selfx_in = nc.dram_tensor(
        "selfx", [NBLK * P, P], f16, kind="ExternalInput"
    ).ap()
    out = nc.dram_tensor("out", [cfg.NPC, DOUT], f32, kind="ExternalOutput").ap()
    xint = nc.dram_tensor("xint", [cfg.NPAD, P], f16, kind="Internal").ap()

    with tile.TileContext(nc) as tc:
        with ExitStack() as ctx:
            cpool = ctx.enter_context(tc.tile_pool(name="const", bufs=1))
            iota_sb = cpool.tile([P, P], f16)
            ident_sb = cpool.tile([P, P], f16)
            w_sb = cpool.tile([P, WH], f16)
            bias_sb = cpool.tile([P, 1], f32)
            dlocv = cpool.tile([P, nt], f16)
            hidx = cpool.tile([P, NGI], i16)
            ev = cpool.tile([P, nt * H], f32)     # exp(lrelu(z)-C)
            ev16 = cpool.tile([P, nt * H], f16)
            nc.sync.dma_start(iota_sb[:], iota_in[:, :])
            nc.sync.dma_start(ident_sb[:], ident_in[:, :])
            nc.sync.dma_start(w_sb[:], w_in[:, :])
            nc.sync.dma_start(bias_sb[:], bias_in[:, :])
            nc.sync.dma_start(dlocv[:], dlocv_in[:, :])
            nc.sync.dma_start(hidx[:], hidx_in[:, :])

            # ---------- Phase 0: stage x-table + batched edge math ----------
            with tc.tile_pool(name="zmath", bufs=1) as zpool:
                as_sb = zpool.tile([P, nt * H], f32)
                ad_sb = zpool.tile([P, nt * H], f32)
                e2 = zpool.tile([P, nt * H], f32)
                nc.sync.dma_start(as_sb[:], as_in[:, :])
                nc.sync.dma_start(ad_sb[:], ad_in[:, :])
                with tc.tile_pool(name="stage", bufs=4) as spool:
                    for i in range(cfg.NTILE_N):
                        st = spool.tile([P, P], f16)
                        nc.sync.dma_start(st[:], xtab_in[i * P : (i + 1) * P, :])
                        nc.sync.dma_start(xint[i * P : (i + 1) * P, :], st[:])
                # z = as + ad (in place into as_sb)
                nc.vector.tensor_add(as_sb[:], as_sb[:], ad_sb[:])
                nc.scalar.activation(ev[:], as_sb[:], AF.Exp, bias=bias_sb[:])
                nc.scalar.activation(
                    e2[:], as_sb[:], AF.Exp, bias=bias_sb[:], scale=SLOPE
                )
                nc.vector.tensor_tensor(
                    out=ev[:], in0=ev[:], in1=e2[:], op=mybir.AluOpType.max
                )
                nc.vector.tensor_copy(ev16[:], ev[:])

            tc.strict_bb_all_engine_barrier()

            # ---------- Phase 1: edge tiles ----------
            xintA = xint[0 : cfg.NSPLIT, :]
            xintB = xint[cfg.NSPLIT :, :]

            def gcall(gh_tile, width_idx, col0, ncols, half):
                nc.gpsimd.dma_gather(
                    out_ap=gh_tile[:, 0 : width_idx * P].rearrange(
                        "p (k e) -> p k e", e=P
                    ),
                    in_ap=xintA if half == 0 else xintB,
                    idxs_ap=hidx[:, col0 : col0 + ncols],
                    num_idxs=width_idx * P,
                    num_idxs_reg=width_idx * P,
                    elem_size=P,
                    single_packet=False,
                )

            with (
                tc.tile_pool(name="gh", bufs=4) as gh_pool,
                tc.tile_pool(name="sx", bufs=3) as sx_pool,
                tc.tile_pool(name="s01", bufs=4) as s01_pool,
                tc.tile_pool(name="xw", bufs=4) as xw_pool,
                tc.tile_pool(name="tl", bufs=4) as tl_pool,
                tc.tile_pool(name="ag", bufs=3) as ag_pool,
                tc.tile_pool(name="ob", bufs=3) as ob_pool,
                tc.tile_pool(name="pso", bufs=2, space="PSUM") as pso_pool,
                tc.tile_pool(name="psd", bufs=2, space="PSUM") as psd_pool,
                tc.tile_pool(name="pst", bufs=2, space="PSUM") as pst_pool,
                tc.tile_pool(name="psf", bufs=2, space="PSUM") as psf_pool,
            ):
                for bp in range(NPAIR + 1):
                    last = bp == NPAIR          # leftover single block
                    blocks = [NBLK - 1] if last else [2 * bp, 2 * bp + 1]
                    ghs = []
                    for h in range(2):
                        g = gh_pool.tile([P, NIH2], f16)
                        if last:
                            col0 = (NPAIR * NIH2 * 2 + h * NIH1) // 16
                            gcall(g, t_half, col0, NIH1 // 16, h)
                        else:
                            col0 = (bp * NIH2 * 2 + h * NIH2) // 16
                            gcall(g, 2 * t_half, col0, NIH2 // 16, h)
                        ghs.append(g)
                    for bi, b in enumerate(blocks):
                        psum_agg = pso_pool.tile([P, WH], f32, space="PSUM")
                        psum_den = psd_pool.tile([P, H], f32, space="PSUM")
                        t0 = b * TB
                        # --- self-loop tile (S01 = identity, sequential x) ---
                        xself = sx_pool.tile([P, P], f16)
                        nc.sync.dma_start(
                            xself[:], selfx_in[b * P : (b + 1) * P, :]
                        )
                        xw = xw_pool.tile([P, WH], f16)
                        nc.vector.tensor_tensor(
                            out=xw[:].rearrange("p (k c) -> p k c", c=P),
                            in0=xself[:]
                            .rearrange("p (k c) -> p k c", k=1)
                            .to_broadcast([P, H, P]),
                            in1=ev[:, t0 * H : (t0 + 1) * H].to_broadcast(
                                [P, H, P]
                            ),
                            op=mybir.AluOpType.mult,
                        )
                        nc.tensor.matmul(
                            psum_agg[:], lhsT=ident_sb[:], rhs=xw[:],
                            start=True, stop=False,
                        )
                        nc.tensor.matmul(
                            psum_den[:], lhsT=ident_sb[:],
                            rhs=ev16[:, t0 * H : (t0 + 1) * H],
                            start=True, stop=False,
                        )
                        # --- gathered tiles ---
                        for h in range(2):
                            for s in range(t_half):
                                t = t0 + 1 + h * t_half + s
                                goff = (bi * t_half + s) * P if not last else s * P
                                xs = ghs[h][:, goff : goff + P]
                                s01 = s01_pool.tile([P, P], f16)
                                nc.vector.tensor_tensor(
                                    out=s01[:],
                                    in0=dlocv[:, t : t + 1].to_broadcast([P, P]),
                                    in1=iota_sb[:],
                                    op=mybir.AluOpType.is_equal,
                                )
                                xw = xw_pool.tile([P, WH], f16)
                                nc.vector.tensor_tensor(
                                    out=xw[:].rearrange("p (k c) -> p k c", c=P),
                                    in0=xs.rearrange(
                                        "p (k c) -> p k c", k=1
                                    ).to_broadcast([P, H, P]),
                                    in1=ev[:, t * H : (t + 1) * H].to_broadcast(
                                        [P, H, P]
                                    ),
                                    op=mybir.AluOpType.mult,
                                )
                                lastmm = h == 1 and s == t_half - 1
                                nc.tensor.matmul(
                                    psum_agg[:], lhsT=s01[:], rhs=xw[:],
                                    start=False, stop=lastmm,
                                )
                                nc.tensor.matmul(
                                    psum_den[:], lhsT=s01[:],
                                    rhs=ev16[:, t * H : (t + 1) * H],
                                    start=False, stop=lastmm,
                                )
                        # --- tail: normalize, transpose, @W, head-sum ---
                        den4 = tl_pool.tile([P, H], f32, tag="den4")
                        nc.vector.tensor_scalar_mul(den4[:], psum_den[:], float(H))
                        rec = tl_pool.tile([P, H], f32, tag="rec")
                        nc.vector.reciprocal(rec[:], den4[:])
                        aggn = ag_pool.tile([P, WH], f16)
                        nc.vector.tensor_tensor(
                            out=aggn[:].rearrange("p (k c) -> p k c", c=P),
                            in0=psum_agg[:].rearrange("p (k c) -> p k c", c=P),
                            in1=rec[:].to_broadcast([P, H, P]),
                            op=mybir.AluOpType.mult,
                        )
                        psum_t = pst_pool.tile([P, WH], f16, space="PSUM")
                        for hd in range(H):
                            nc.tensor.transpose(
                                psum_t[:, hd * P : (hd + 1) * P],
                                aggn[:, hd * P : (hd + 1) * P],
                                ident_sb[:],
                            )
                        aggT = ag_pool.tile([P, WH], f16, tag="aggT")
                        nc.vector.tensor_copy(aggT[:], psum_t[:])
                        psum_o = psf_pool.tile([P, DOUT], f32, space="PSUM")
                        for hd in range(H):
                            nc.tensor.matmul(
                                psum_o[:],
                                lhsT=aggT[:, hd * P : (hd + 1) * P],
                                rhs=w_sb[:, hd * P : (hd + 1) * P],
                                start=hd == 0,
                                stop=hd == H - 1,
                            )
                        osb = ob_pool.tile([P, DOUT], f32)
                        nc.vector.tensor_copy(osb[:], psum_o[:])
                        rows = cfg.LAST_ROWS if b == NBLK - 1 else P
                        nc.sync.dma_start(
                            out[b * P : b * P + rows, :], osb[:rows, :]
                        )

    nc.compile()
    return nc


def _wrap16(idx_flat, ni_per_group):
    """[G*NI] gather indices -> [128, G*NI/16] wrapped-16 layout (x8 groups)."""
    g = idx_flat.reshape(-1, ni_per_group)
    ng = g.shape[0]
    w = np.zeros((16, ng, ni_per_group // 16), np.int16)
    for p in range(16):
        w[p] = g[:, p::16]
    w = w.reshape(16, ng * (ni_per_group // 16))
    return np.tile(w, (8, 1))


def _prep(cfg: Cfg, x, edge_index, W, att_src, att_dst):
    """Host-side sharding/layout -> (per-core in_maps, t_half)."""
    f16 = np.float16
    N, H, DIN, DOUT = cfg.N, cfg.H, cfg.DIN, cfg.DOUT
    NBLK, NPC = cfg.NBLK, cfg.NPC
    x = np.asarray(x, np.float32)
    Wn = np.asarray(W, np.float32)
    src = np.asarray(edge_index[0]).astype(np.int64)
    dst = np.asarray(edge_index[1]).astype(np.int64)

    # per-node attention projections (tiny host matmul: x @ (W @ att))
    ws = np.einsum("khc,hc->kh", Wn.reshape(DIN, H, DOUT), np.asarray(att_src, np.float32))
    wd = np.einsum("khc,hc->kh", Wn.reshape(DIN, H, DOUT), np.asarray(att_dst, np.float32))
    as_n = x @ ws                                   # [N, H] f32
    ad_n = x @ wd

    # softmax shift constant: ev = exp(lrelu(z) - C) must fit f16 comfortably
    z_all = as_n[src] + ad_n[dst]
    z_self = as_n + ad_n
    lr = lambda z: np.where(z > 0, z, SLOPE * z)
    zmax = max(float(lr(z_all).max()), float(lr(z_self).max()))
    C = zmax - 8.0
    bias = np.full((P, 1), -C, np.float32)

    # sort non-self edges by (core, block, src-half)
    core = dst // NPC
    ln = dst - core * NPC
    half = (src >= cfg.NSPLIT).astype(np.int64)
    key = (core * NBLK + ln // P) * 2 + half
    order = np.argsort(key, kind="stable")
    src_s = src[order].astype(np.int32)
    ln_s = ln[order].astype(np.int32)
    zsrc_s = as_n[src[order]].astype(np.float32)    # [Es, H]
    zdst_s = ad_n[dst[order]].astype(np.float32)
    key_s = key[order]

    nseg = cfg.NCORES * NBLK * 2
    counts = np.bincount(key_s, minlength=nseg)
    t_half = int(max(1, ((counts + P - 1) // P).max()))
    TB = 1 + 2 * t_half
    nt = NBLK * TB
    starts = np.concatenate([[0], np.cumsum(counts)])

    # flat per-core layouts
    hsrc = np.zeros((cfg.NCORES, (nt - NBLK) * P), np.int16)   # gather idx
    dloc = np.full((cfg.NCORES, nt * P), -1.0, f16)
    as_e = np.full((cfg.NCORES, nt * P, H), ZPAD, np.float32)
    ad_e = np.zeros((cfg.NCORES, nt * P, H), np.float32)

    # self tiles: tile b*TB + 0, partition p = node c*NPC + min(b*128+p, NPC-1)
    for c in range(cfg.NCORES):
        for b in range(NBLK):
            t0 = (b * TB) * P
            gids = c * NPC + np.minimum(b * P + np.arange(P), NPC - 1)
            as_e[c, t0 : t0 + P] = as_n[gids]
            ad_e[c, t0 : t0 + P] = ad_n[gids]
            # dloc unused for self tiles (identity lhsT)
    # regular tiles
    for c in range(cfg.NCORES):
        for b in range(NBLK):
            for hh in range(2):
                seg = (c * NBLK + b) * 2 + hh
                s, e = starts[seg], starts[seg + 1]
                cnt = e - s
                t = b * TB + 1 + hh * t_half       # first tile of this segment
                o = t * P                          # slot offset in nt*P space
                og = (b * 2 * t_half + hh * t_half) * P  # offset in gather space
                hsrc[c, og : og + cnt] = (
                    src_s[s:e] - (cfg.NSPLIT if hh else 0)
                ).astype(np.int16)
                dloc[c, o : o + cnt] = (ln_s[s:e] - b * P).astype(f16)
                as_e[c, o : o + cnt] = zsrc_s[s:e]
                ad_e[c, o : o + cnt] = zdst_s[s:e]

    # reorder gather idx into call groups: pairs (bp, half) then leftover
    NIH2 = 2 * t_half * P
    xpad = np.zeros((cfg.NPAD, DIN), np.float32)
    xpad[:N] = x
    xtab = xpad.astype(f16)
    iota = np.broadcast_to(np.arange(P, dtype=f16), (P, P)).copy()
    ident = np.eye(P, dtype=f16)
    wfin = Wn.astype(f16)

    in_maps = []
    for c in range(cfg.NCORES):
        hs = hsrc[c].reshape(NBLK, 2, t_half * P)   # [block, half, slot]
        groups = []
        for bp in range(NBLK // 2):
            for h in range(2):
                groups.append(
                    np.concatenate([hs[2 * bp, h], hs[2 * bp + 1, h]])
                )
        gidx = [_wrap16(np.concatenate(groups), NIH2)]
        for h in range(2):                           # leftover block
            gidx.append(_wrap16(hs[NBLK - 1, h], t_half * P))
        in_maps.append(
            {
                "xtab": xtab,
                "iota": iota,
                "ident": ident,
                "w": wfin,
                "bias": bias,
                "ase": np.ascontiguousarray(
                    as_e[c].reshape(nt, P, H).transpose(1, 0, 2).reshape(P, nt * H)
                ),
                "ade": np.ascontiguousarray(
                    ad_e[c].reshape(nt, P, H).transpose(1, 0, 2).reshape(P, nt * H)
                ),
                "hidx": np.concatenate(gidx, axis=1),
                "dlocv": np.ascontiguousarray(dloc[c].reshape(nt, P).T),
                "selfx": xtab[
                    c * NPC
                    + np.minimum(
                        np.arange(NBLK * P), NPC - 1
                    )
                ],
            }
        )
    return in_maps, t_half


def run(cfg: Cfg, x, edge_index, W, att_src, att_dst, trace=False, sim=False,
        sim_cores=None):
    in_maps, t_half = _prep(cfg, x, edge_index, W, att_src, att_dst)
    nc = _build_program(cfg, t_half)
    if sim:
        from concourse.bass_interp import CoreSim

        outs = []
        for c in sim_cores if sim_cores is not None else range(cfg.NCORES):
            s = CoreSim(nc, trace=False, require_finite=False, require_nnan=False)
            for k, v in in_maps[c].items():
                s.tensor(k)[:] = v
            s.simulate(check_with_hw=False)
            outs.append(np.array(s.tensor("out")))
        return np.concatenate(outs, axis=0), None
    from concourse.bass_utils import run_bass_kernel_spmd

    res = run_bass_kernel_spmd(
        nc, in_maps, core_ids=list(range(cfg.NCORES)), trace=trace
    )
    out = np.concatenate([r["out"] for r in res.results], axis=0)
    return out.astype(np.float32), res


def kernel(x, edge_index, W, att_src, att_dst):
    x = np.asarray(x)
    edge_index = np.asarray(edge_index)
    out, _ = run(DEFAULT_CFG, x, edge_index, W, att_src, att_dst)
    return out
